# revision 20
# baseline (speedup 1.0000x reference)
"""Trainium2 Bass kernel for nn_DSCAMSFF (1x1 conv + per-group CBAM gating).

Only x4 is live in the reference model (cov1-3 / the attention path are dead
code that returns its first argument). Effective computation per batch b:

  a  = conv1x1(x4[b]) : [512, 256]          (w [512,2048], pixels flattened)
  x  = concat([a]*4)  : [2048, 256] in 8 groups of 256 channels
  per group g (channels of group g are a[(g%2)*256 : (g%2+1)*256]):
    avg_g = mean_px(a_g)                       [256]
    h_g   = relu(fc1_w[g] @ avg_g + fc1_b[g])  [64]
    ca_g  = sigmoid(fc2_w[g] @ h_g + fc2_b[g]) [256]
    sa_g  = sigmoid((ca_g*sa_w[g]) . a_g + sa_b[g])   [256 px]
    z_g   = sigmoid(a_g * ca_g[:,None] * sa_g[None,:])
    mean_g = mean(z_g)
    mask  = where(z_g > mean_g, 1, z_g)
    out_g = a_g * (mask + 1)

Sharding: pure data-parallel over batch (8 cores x 1 batch element),
parameters replicated. All matmuls run in fp16 (fp32 PSUM accumulate); all
biases are folded into K=1 matmuls so activations run as wide bias-free ops;
the mask arithmetic and the final multiply stay fp32.
"""

import numpy as np

N_CORES = 8
P = 128
PX = 256            # 16*16 pixels
KT = 16             # 2048 / 128 K tiles
MT = 4              # 512 / 128 conv out tiles

# fp16 packed tensor layout (columns); first half is DMA'd separately so the
# conv/fc1 parameters land before the fc2 block
_CB_OFF = 0         # conv bias rows  [m, mc] 4*128 = 512 (partition 0 only)
_W1_OFF = 512       # [p, kt, mm]     2*2*256  = 1024
_FB_OFF = 1536      # fc1 bias rows   [p, mt, m] 512   (partition 0 only)
_W2_OFF = 2048      # [p, i, s, m]    2*4*2*128 = 2048 (with bias/ones rows)
_NSM16 = 4096
# fp32 packed params
_SAW_OFF = 0        # [p, s, i] 16
_SAB_OFF = 16       # [g] 8
_NSM32 = 24

_CACHE = {}


def _register_dve_ops():
    """Register the two fused DVE ops (idempotent, runtime-only)."""
    from concourse import dve_ops as DO
    from concourse.dve_spec import Spec, Src0, Src1, C0, One, select, lower
    from concourse.dve_uop import DveOpSpec

    if "DSCAM_MASK_MUL" in DO._SUB_OPCODE_FOR_NAME:
        by = {o.name: o for o in DO.OPS}
        return by["DSCAM_TSA_MUL"], by["DSCAM_MASK_MUL"]

    def mk(name, spec):
        row = DO._CUSTOM_DVE_ROW_BASE + len(DO.OPS)
        DO._SUB_OPCODE_FOR_NAME[name] = row
        shas = {}
        for ver in ("v3", "v4"):
            try:
                uops = lower(spec, ver=ver)
                shas[ver] = DveOpSpec(name=name, opcode=row, uops=uops,
                                      rd1_en=True).sha(ver)
            except Exception:
                pass
        op = DO.DveOp(name, spec, subdim=False, uops_sha=shas)
        DO.OPS.append(op)
        DO.CUSTOM_DVE_SPECS[name] = spec
        return op

    tsa = mk("DSCAM_TSA_MUL", Spec(
        body=Src0 * Src1 * C0,
        reference=lambda in0, in1, s0, s1, imm2:
            (in0.astype(np.float32) * in1 * s0).astype(np.float32),
    ))
    msk = mk("DSCAM_MASK_MUL", Spec(
        body=Src1 * (One + select(Src0 > C0, One, Src0)),
        reference=lambda in0, in1, s0, s1, imm2:
            (in1 * (1.0 + np.where(in0 > s0, 1.0, in0))).astype(np.float32),
    ))
    return tsa, msk


def _build_program():
    import concourse.mybir as mybir
    import concourse.tile as tile
    from concourse import bacc, bass_isa

    fp32 = mybir.dt.float32
    fp16 = mybir.dt.float16
    Act = mybir.ActivationFunctionType
    Alu = mybir.AluOpType
    AX = mybir.AxisListType

    _TSA_OP, _MSK_OP = _register_dve_ops()

    nc = bacc.Bacc("TRN2", target_bir_lowering=False, debug=False)

    x_d = nc.dram_tensor("x", [P, KT, PX], fp16, kind="ExternalInput").ap()
    w_d = nc.dram_tensor("w", [MT, P, KT, P], fp16, kind="ExternalInput").ap()
    s16_d = nc.dram_tensor("s16", [P, _NSM16], fp16, kind="ExternalInput").ap()
    s32_d = nc.dram_tensor("s32", [P, _NSM32], fp32, kind="ExternalInput").ap()
    cb_d = nc.dram_tensor("cb", [1, 512], fp16, kind="ExternalInput").ap()
    out_d = nc.dram_tensor("out", [16, P, PX], fp32, kind="ExternalOutput").ap()

    with tile.TileContext(nc) as tc:
        with (
            tc.tile_pool(name="singles", bufs=1) as singles,
            tc.tile_pool(name="work", bufs=4) as work,
            tc.tile_pool(name="zpool", bufs=5) as zpool,
            tc.tile_pool(name="otp", bufs=6) as otp,
            tc.tile_pool(name="psA", bufs=2, space="PSUM") as psA,
            tc.tile_pool(name="psB", bufs=2, space="PSUM") as psB,
        ):
            # ---- input DMAs: two issue queues, ordered by need time ----
            xta = singles.tile([P, 8, PX], fp16, tag="xta")
            xtb = singles.tile([P, 8, PX], fp16, tag="xtb")
            wt = [None] * MT
            for m in range(MT):
                wt[m] = singles.tile([P, KT, P], fp16, tag=f"w{m}", name=f"w{m}")
            s16 = singles.tile([P, _NSM16], fp16, tag="s16")
            s32 = singles.tile([P, _NSM32], fp32, tag="s32")

            cb_sb = singles.tile([1, 512], fp16, tag="cb_sb")
            nc.sync.dma_start(out=wt[0], in_=w_d[0])
            nc.sync.dma_start(out=cb_sb, in_=cb_d)
            nc.sync.dma_start(out=xta, in_=x_d[:, :8, :])
            nc.sync.dma_start(out=xtb, in_=x_d[:, 8:, :])
            nc.sync.dma_start(out=wt[1], in_=w_d[1])
            nc.sync.dma_start(out=s16[:, _W1_OFF:_W2_OFF],
                              in_=s16_d[:, _W1_OFF:_W2_OFF])
            nc.sync.dma_start(out=wt[2], in_=w_d[2])
            nc.sync.dma_start(out=wt[3], in_=w_d[3])
            nc.sync.dma_start(out=s16[:, _W2_OFF:], in_=s16_d[:, _W2_OFF:])
            nc.sync.dma_start(out=s32, in_=s32_d)

            # parameter views
            w1v = s16[:, _W1_OFF:_W1_OFF + 1024].rearrange(
                "P (p k m) -> P p k m", p=2, k=2)
            w2v = s16[:, _W2_OFF:_W2_OFF + 2048].rearrange(
                "P (p i s m) -> P p i s m", p=2, i=4, s=2)
            cbias = cb_sb.rearrange("o (m c) -> o m c", m=4)
            fbias = s16[0:1, _FB_OFF:_FB_OFF + 512].rearrange(
                "o (p t c) -> o p t c", p=2, t=2)
            sawv = s32[:, _SAW_OFF:_SAW_OFF + 16].rearrange(
                "P (p s i) -> P p s i", p=2, s=2)
            sabv = s32[:, _SAB_OFF:_SAB_OFF + 8]

            # constants
            ones16 = singles.tile([1, PX], fp16, tag="ones16")
            nc.gpsimd.memset(ones16, 1.0)
            onesPK = singles.tile([P, PX], fp16, tag="onesPK")
            nc.gpsimd.memset(onesPK, 1.0)
            ones32 = singles.tile([P, P], fp32, tag="ones32")
            nc.gpsimd.memset(ones32, 1.0)
            # per-group spatial bias rows (replicated to 128 cols, fp16)
            sab16 = singles.tile([1, 8, P], fp16, tag="sab16")
            nc.vector.tensor_copy(
                out=sab16, in_=sabv[0:1, :, None].to_broadcast((1, 8, P)))

            # ACT table preload + PE HAM warmup while input DMAs stream
            tl = singles.tile([1, 1], fp32, tag="tl")
            nc.scalar.activation(out=tl, in_=ones16[:, 0:1], func=Act.Sigmoid)
            for wu in range(30):
                wps = psA.tile([P, PX], fp32, tag="sa")
                nc.tensor.matmul(wps, lhsT=onesPK[:, 0:P], rhs=onesPK,
                                 start=True, stop=True)

            a_sb = [None, None]
            a16 = [None, None]
            asum16 = [None, None]
            h_sb = [None, None]
            h_m = [None, None]
            ca = [None, None]
            weff16 = [None, None]

            def conv_p(p):
                a_sb[p] = singles.tile([P, 2, PX], fp32, tag=f"a{p}", name=f"a{p}")
                a16[p] = singles.tile([P, 2, PX], fp16, tag=f"a16_{p}",
                                      name=f"a16_{p}")
                asum = work.tile([P, 2], fp32, tag="asum")
                for s in (0, 1):
                    m = 2 * p + s
                    ps = psA.tile([P, PX], fp32, tag="conv")
                    for kt in range(KT):
                        xsrc = xta if kt < 8 else xtb
                        nc.tensor.matmul(
                            ps, lhsT=wt[m][:, kt, :],
                            rhs=xsrc[:, kt % 8, :],
                            start=(kt == 0), stop=False)
                    nc.tensor.matmul(
                        ps, lhsT=cbias[:, m, :], rhs=ones16,
                        start=False, stop=True)
                    nc.scalar.copy(out=a_sb[p][:, s, :], in_=ps)
                    nc.vector.tensor_copy(out=a16[p][:, s, :], in_=ps)
                    nc.vector.tensor_reduce(asum[:, s:s + 1], ps, axis=AX.X,
                                            op=Alu.add)
                asum16[p] = singles.tile([P, 2], fp16, tag=f"as16_{p}",
                                         name=f"as16_{p}")
                # 1/256 pixel-mean folded here (not into fp16 weights:
                # 0.02/256 would be subnormal in fp16)
                nc.vector.tensor_scalar_mul(asum16[p], asum, 1.0 / 256.0)

            def fc_chain(p):
                # fc1: h = relu(W1 @ avg + b1), 4 groups of 64 stacked
                hp = psB.tile([P, 2], fp32, tag="tiny8")
                for mt in (0, 1):
                    for kt in (0, 1):
                        nc.tensor.matmul(
                            hp[:, mt:mt + 1],
                            lhsT=w1v[:, p, kt, mt * P:(mt + 1) * P],
                            rhs=asum16[p][:, kt:kt + 1],
                            start=(kt == 0), stop=False)
                    nc.tensor.matmul(
                        hp[:, mt:mt + 1], lhsT=fbias[:, p, mt, :],
                        rhs=ones16[:, 0:1], start=False, stop=True)
                h_sb[p] = singles.tile([P, 2], fp16, tag=f"h{p}", name=f"h{p}")
                nc.scalar.activation(out=h_sb[p], in_=hp, func=Act.Relu)
                # per-group h with a 1.0 row so fc2's bias folds into the
                # matmul: even groups keep h in rows 0-63 (ones at row 64),
                # odd groups in rows 64-127 (ones at row 0)
                h_m[p] = singles.tile([P, 4], fp16, tag=f"hm{p}", name=f"hm{p}")
                nc.vector.memset(h_m[p], 0.0)
                for i in range(4):
                    lo = 64 * (i % 2)
                    nc.vector.tensor_copy(
                        out=h_m[p][lo:lo + 64, i:i + 1],
                        in_=h_sb[p][lo:lo + 64, i // 2:i // 2 + 1])
                    one_row = 64 if i % 2 == 0 else 0
                    nc.vector.memset(h_m[p][one_row:one_row + 1, i:i + 1], 1.0)
                # fc2: ca = sigmoid(W2 @ h + b2), all 8 columns in one bank
                cp = psB.tile([P, 2, 4], fp32, tag="tiny8")
                for s in (0, 1):
                    for i in range(4):
                        nc.tensor.matmul(
                            cp[:, s, i:i + 1], lhsT=w2v[:, p, i, s, :],
                            rhs=h_m[p][:, i:i + 1], start=True, stop=True)
                ca[p] = singles.tile([P, 2, 4], fp32, tag=f"ca{p}", name=f"ca{p}")
                nc.scalar.activation(out=ca[p], in_=cp, func=Act.Sigmoid)
                weff16[p] = singles.tile([P, 2, 4], fp16, tag=f"we{p}",
                                         name=f"we{p}")
                nc.vector.tensor_tensor(out=weff16[p], in0=ca[p],
                                        in1=sawv[:, p], op=Alu.mult)

            def saz_p(p, zs, zsum):
                sarep = [None, None]
                for i in range(4):
                    g = p + 2 * i
                    j = i % 2
                    if j == 0:
                        sarep[i // 2] = work.tile([P, 2, PX], fp16, tag="sarep",
                                                  name="sarep")
                        sps = psA.tile([P, 2, PX], fp32, tag="sa")
                    # rank-1 "broadcast" matmuls: every output partition gets
                    # the same spatial-sum row; bias via a K=1 matmul
                    for s in (0, 1):
                        nc.tensor.matmul(
                            sps[:, j, :],
                            lhsT=weff16[p][:, s, i:i + 1].to_broadcast((P, P)),
                            rhs=a16[p][:, s, :],
                            start=(s == 0), stop=False)
                    nc.tensor.matmul(
                        sps[:, j, :], lhsT=sab16[:, g, :], rhs=ones16,
                        start=False, stop=True)
                    if j == 1:
                        nc.scalar.activation(out=sarep[i // 2], in_=sps,
                                             func=Act.Sigmoid)
                for i in range(4):
                    t16 = work.tile([P, 2, PX], fp16, tag="t16")
                    for s in (0, 1):
                        # fused t = a * sa * ca in one DVE pass
                        nc.vector._custom_dve(
                            _TSA_OP, out=t16[:, s, :], in0=a16[p][:, s, :],
                            in1=sarep[i // 2][:, i % 2, :],
                            s0=ca[p][:, s, i:i + 1])
                    z_pair = zpool.tile([P, 2, PX], fp32, tag="z")
                    nc.scalar.activation(
                        out=z_pair, in_=t16, func=Act.Sigmoid,
                        accum_out=zsum[:, i:i + 1])
                    zs[i] = z_pair

            def mask_p(p, zs, zsum):
                pm = singles.tile([P, 4], fp32, tag=f"pm{p}", name=f"pm{p}")
                for i in range(4):
                    # per-group cross-partition sum via an all-ones fp32
                    # matmul: every output partition gets the group total
                    zrps = psB.tile([P, 2, 4], fp32, tag="tiny8")
                    nc.tensor.matmul(zrps[:, 0, 0:1], lhsT=ones32,
                                     rhs=zsum[:, i:i + 1], start=True, stop=True)
                    nc.vector.tensor_scalar_mul(
                        pm[:, i:i + 1], zrps[:, 0, 0:1], 1.0 / 65536.0)
                    ot = otp.tile([P, 2, PX], fp32, tag="ot")
                    # fused out = a * (1 + where(z > mean, 1, z))
                    nc.vector._custom_dve(
                        _MSK_OP, out=ot, in0=zs[i], in1=a_sb[p],
                        s0=pm[:, i:i + 1])
                    nc.sync.dma_start(
                        out=out_d.rearrange("(i x) P f -> P i x f", i=4)
                        [:, i, 2 * p:2 * p + 2, :],
                        in_=ot)

            zs0, zs1 = {}, {}
            zsum0 = singles.tile([P, 4], fp32, tag="zs0")
            zsum1 = singles.tile([P, 4], fp32, tag="zs1")
            conv_p(0)
            fc_chain(0)
            conv_p(1)
            fc_chain(1)
            saz_p(0, zs0, zsum0)
            saz_p(1, zs1, zsum1)
            mask_p(0, zs0, zsum0)
            mask_p(1, zs1, zsum1)

    nc.finalize()
    return nc


def _prep_core_inputs(x4b, w, s16, s32, cb):
    x = np.ascontiguousarray(
        x4b.reshape(KT, P, PX).transpose(1, 0, 2)).astype(np.float16)
    return {"x": x, "w": w, "s16": s16, "s32": s32, "cb": cb}


def _prep_params(cov4_w, cov4_b, fc1_w, fc1_b, fc2_w, fc2_b, sa_w, sa_b):
    f32 = np.float32
    w2d = np.asarray(cov4_w, f32).reshape(512, 2048)
    wr = w2d.reshape(MT, P, KT, P)                            # [m, mc, kt, part]
    w_arr = np.ascontiguousarray(wr.transpose(0, 3, 2, 1)).astype(np.float16)

    fc1_w = np.asarray(fc1_w, f32)
    fc1_b = np.asarray(fc1_b, f32)
    fc2_w = np.asarray(fc2_w, f32)
    fc2_b = np.asarray(fc2_b, f32)
    sa_w = np.asarray(sa_w, f32)
    sa_b = np.asarray(sa_b, f32)

    w1 = np.zeros((P, 2, 2, 256), f32)
    w2 = np.zeros((P, 2, 4, 2, P), f32)
    b1 = np.zeros((2, 2, P), f32)
    saw = np.zeros((P, 2, 2, 4), f32)
    for p in range(2):
        W1s = np.concatenate([fc1_w[p + 2 * i] for i in range(4)], axis=0)
        b1s = np.concatenate([fc1_b[p + 2 * i] for i in range(4)], axis=0)
        for kt in range(2):
            w1[:, p, kt, :] = W1s[:, kt * P:(kt + 1) * P].T
        b1[p, 0] = b1s[:P]
        b1[p, 1] = b1s[P:]
        for i in range(4):
            g = p + 2 * i
            lo = 64 * (i % 2)           # rows holding fc2 weights
            brow = 64 if i % 2 == 0 else 0
            for s in range(2):
                w2[lo:lo + 64, p, i, s, :] = fc2_w[g][s * P:(s + 1) * P, :].T
                w2[brow, p, i, s, :] = fc2_b[g, s * P:(s + 1) * P]
                saw[:, p, s, i] = sa_w[g, s * P:(s + 1) * P]

    s16 = np.zeros((P, _NSM16), np.float16)
    s16[:, _W1_OFF:_W1_OFF + 1024] = w1.reshape(P, 1024).astype(np.float16)
    s16[:, _W2_OFF:_W2_OFF + 2048] = w2.reshape(P, 2048).astype(np.float16)
    s16[0, _FB_OFF:_FB_OFF + 512] = b1.reshape(512).astype(np.float16)
    cb = np.asarray(cov4_b, f32).astype(np.float16).reshape(1, 512)

    s32 = np.zeros((P, _NSM32), f32)
    s32[:, _SAW_OFF:_SAW_OFF + 16] = saw.reshape(P, 16)
    s32[:, _SAB_OFF:_SAB_OFF + 8] = np.broadcast_to(sa_b, (P, 8))
    return w_arr, s16, s32, cb


def kernel(**inputs):
    from concourse.bass_utils import run_bass_kernel_spmd

    if "nc" not in _CACHE:
        _CACHE["nc"] = _build_program()
    nc = _CACHE["nc"]

    x4 = np.asarray(inputs["x4"], np.float32)
    B = x4.shape[0]
    w_arr, s16, s32, cb = _prep_params(
        inputs["cov4_w"], inputs["cov4_b"],
        inputs["gce_fc1_w"], inputs["gce_fc1_b"],
        inputs["gce_fc2_w"], inputs["gce_fc2_b"],
        inputs["gce_sa_w"], inputs["gce_sa_b"])

    in_maps = [
        _prep_core_inputs(x4[b].reshape(2048, PX), w_arr, s16, s32, cb)
        for b in range(B)
    ]
    res = run_bass_kernel_spmd(nc, in_maps, list(range(N_CORES)))
    _CACHE["last_results"] = res

    out = np.empty((B, 2048, 16, 16), np.float32)
    for b in range(B):
        out[b] = res.results[b]["out"].reshape(2048, 16, 16)
    return out


# revision 21
# speedup vs baseline: 1.0652x; 1.0652x over previous
"""Trainium2 Bass kernel for nn_DSCAMSFF (1x1 conv + per-group CBAM gating).

Only x4 is live in the reference model (cov1-3 / the attention path are dead
code that returns its first argument). Effective computation per batch b:

  a  = conv1x1(x4[b]) : [512, 256]          (w [512,2048], pixels flattened)
  x  = concat([a]*4)  : [2048, 256] in 8 groups of 256 channels
  per group g (channels of group g are a[(g%2)*256 : (g%2+1)*256]):
    avg_g = mean_px(a_g)                       [256]
    h_g   = relu(fc1_w[g] @ avg_g + fc1_b[g])  [64]
    ca_g  = sigmoid(fc2_w[g] @ h_g + fc2_b[g]) [256]
    sa_g  = sigmoid((ca_g*sa_w[g]) . a_g + sa_b[g])   [256 px]
    z_g   = sigmoid(a_g * ca_g[:,None] * sa_g[None,:])
    mean_g = mean(z_g)
    mask  = where(z_g > mean_g, 1, z_g)
    out_g = a_g * (mask + 1)

Sharding: pure data-parallel over batch (8 cores x 1 batch element),
parameters replicated. All matmuls run in fp16 (fp32 PSUM accumulate); all
biases are folded into K=1 matmuls so activations run as wide bias-free ops;
the mask arithmetic and the final multiply stay fp32.
"""

import numpy as np

N_CORES = 8
P = 128
PX = 256            # 16*16 pixels
KT = 16             # 2048 / 128 K tiles
MT = 4              # 512 / 128 conv out tiles

# fp16 packed tensor layout (columns); first half is DMA'd separately so the
# conv/fc1 parameters land before the fc2 block
_CB_OFF = 0         # conv bias rows  [m, mc] 4*128 = 512 (partition 0 only)
_W1_OFF = 512       # [p, kt, mm]     2*2*256  = 1024
_FB_OFF = 1536      # fc1 bias rows   [p, mt, m] 512   (partition 0 only)
_W2_OFF = 2048      # [p, i, s, m]    2*4*2*128 = 2048 (with bias/ones rows)
_NSM16 = 4096
# fp32 packed params
_SAW_OFF = 0        # [p, s, i] 16
_SAB_OFF = 16       # [g] 8
_NSM32 = 24

_CACHE = {}


def _register_dve_ops():
    """Register the two fused DVE ops (idempotent, runtime-only)."""
    from concourse import dve_ops as DO
    from concourse.dve_spec import Spec, Src0, Src1, C0, One, select, lower
    from concourse.dve_uop import DveOpSpec

    if "DSCAM_MASK_MUL" in DO._SUB_OPCODE_FOR_NAME:
        by = {o.name: o for o in DO.OPS}
        return by["DSCAM_TSA_MUL"], by["DSCAM_MASK_MUL"]

    def mk(name, spec):
        row = DO._CUSTOM_DVE_ROW_BASE + len(DO.OPS)
        DO._SUB_OPCODE_FOR_NAME[name] = row
        shas = {}
        for ver in ("v3", "v4"):
            try:
                uops = lower(spec, ver=ver)
                shas[ver] = DveOpSpec(name=name, opcode=row, uops=uops,
                                      rd1_en=True).sha(ver)
            except Exception:
                pass
        op = DO.DveOp(name, spec, subdim=False, uops_sha=shas)
        DO.OPS.append(op)
        DO.CUSTOM_DVE_SPECS[name] = spec
        return op

    tsa = mk("DSCAM_TSA_MUL", Spec(
        body=Src0 * Src1 * C0,
        reference=lambda in0, in1, s0, s1, imm2:
            (in0.astype(np.float32) * in1 * s0).astype(np.float32),
    ))
    msk = mk("DSCAM_MASK_MUL", Spec(
        body=Src1 * (One + select(Src0 > C0, One, Src0)),
        reference=lambda in0, in1, s0, s1, imm2:
            (in1 * (1.0 + np.where(in0 > s0, 1.0, in0))).astype(np.float32),
    ))
    return tsa, msk


def _build_program():
    import concourse.mybir as mybir
    import concourse.tile as tile
    from concourse import bacc, bass_isa

    fp32 = mybir.dt.float32
    fp16 = mybir.dt.float16
    Act = mybir.ActivationFunctionType
    Alu = mybir.AluOpType
    AX = mybir.AxisListType

    _TSA_OP, _MSK_OP = _register_dve_ops()

    nc = bacc.Bacc("TRN2", target_bir_lowering=False, debug=False)

    x_d = nc.dram_tensor("x", [P, KT, PX], fp16, kind="ExternalInput").ap()
    w_d = nc.dram_tensor("w", [MT, P, KT, P], fp16, kind="ExternalInput").ap()
    s16_d = nc.dram_tensor("s16", [P, _NSM16], fp16, kind="ExternalInput").ap()
    s32_d = nc.dram_tensor("s32", [P, _NSM32], fp32, kind="ExternalInput").ap()
    cb_d = nc.dram_tensor("cb", [1, 512], fp16, kind="ExternalInput").ap()
    out_d = nc.dram_tensor("out", [16, P, PX], fp32, kind="ExternalOutput").ap()

    with tile.TileContext(nc) as tc:
        with (
            tc.tile_pool(name="singles", bufs=1) as singles,
            tc.tile_pool(name="work", bufs=4) as work,
            tc.tile_pool(name="zpool", bufs=5) as zpool,
            tc.tile_pool(name="otp", bufs=6) as otp,
            tc.tile_pool(name="psA", bufs=2, space="PSUM") as psA,
            tc.tile_pool(name="psB", bufs=2, space="PSUM") as psB,
        ):
            # ---- input DMAs: two issue queues, ordered by need time ----
            xta = singles.tile([P, 8, PX], fp16, tag="xta")
            xtb = singles.tile([P, 8, PX], fp16, tag="xtb")
            wt = [None] * MT
            for m in range(MT):
                wt[m] = singles.tile([P, KT, P], fp16, tag=f"w{m}", name=f"w{m}")
            s16 = singles.tile([P, _NSM16], fp16, tag="s16")
            s32 = singles.tile([P, _NSM32], fp32, tag="s32")

            cb_sb = singles.tile([1, 512], fp16, tag="cb_sb")
            nc.sync.dma_start(out=wt[0], in_=w_d[0])
            nc.sync.dma_start(out=cb_sb, in_=cb_d)
            nc.sync.dma_start(out=xta, in_=x_d[:, :8, :])
            nc.sync.dma_start(out=xtb, in_=x_d[:, 8:, :])
            nc.sync.dma_start(out=wt[1], in_=w_d[1])
            nc.sync.dma_start(out=s16[:, _W1_OFF:_W2_OFF],
                              in_=s16_d[:, _W1_OFF:_W2_OFF])
            nc.sync.dma_start(out=wt[2], in_=w_d[2])
            nc.sync.dma_start(out=wt[3], in_=w_d[3])
            nc.sync.dma_start(out=s16[:, _W2_OFF:], in_=s16_d[:, _W2_OFF:])
            nc.sync.dma_start(out=s32, in_=s32_d)

            # parameter views
            w1v = s16[:, _W1_OFF:_W1_OFF + 1024].rearrange(
                "P (p k m) -> P p k m", p=2, k=2)
            w2v = s16[:, _W2_OFF:_W2_OFF + 2048].rearrange(
                "P (p i s m) -> P p i s m", p=2, i=4, s=2)
            cbias = cb_sb.rearrange("o (m c) -> o m c", m=4)
            fbias = s16[0:1, _FB_OFF:_FB_OFF + 512].rearrange(
                "o (p t c) -> o p t c", p=2, t=2)
            sawv = s32[:, _SAW_OFF:_SAW_OFF + 16].rearrange(
                "P (p s i) -> P p s i", p=2, s=2)
            sabv = s32[:, _SAB_OFF:_SAB_OFF + 8]

            # constants
            ones16 = singles.tile([1, PX], fp16, tag="ones16")
            nc.gpsimd.memset(ones16, 1.0)
            onesPK = singles.tile([P, PX], fp16, tag="onesPK")
            nc.gpsimd.memset(onesPK, 1.0)
            ones32 = singles.tile([P, P], fp32, tag="ones32")
            nc.gpsimd.memset(ones32, 1.0)
            # per-group spatial bias rows (replicated to 128 cols, fp16)
            sab16 = singles.tile([1, 8, P], fp16, tag="sab16")
            nc.vector.tensor_copy(
                out=sab16, in_=sabv[0:1, :, None].to_broadcast((1, 8, P)))

            # ACT table preload + PE HAM warmup while input DMAs stream
            tl = singles.tile([1, 1], fp32, tag="tl")
            nc.scalar.activation(out=tl, in_=ones16[:, 0:1], func=Act.Sigmoid)
            for wu in range(30):
                wps = psA.tile([P, PX], fp32, tag="sa")
                nc.tensor.matmul(wps, lhsT=onesPK[:, 0:P], rhs=onesPK,
                                 start=True, stop=True)

            a_sb = [None, None]
            a16 = [None, None]
            asum16 = [None, None]
            h_sb = [None, None]
            h_m = [None, None]
            ca = [None, None]
            weff16 = [None, None]

            def conv_p(p):
                # two conv out tiles (m = 2p, 2p+1) share one PSUM bank
                ps = psA.tile([P, 2, PX], fp32, tag="conv")
                for s in (0, 1):
                    m = 2 * p + s
                    for kt in range(KT):
                        xsrc = xta if kt < 8 else xtb
                        nc.tensor.matmul(
                            ps[:, s, :], lhsT=wt[m][:, kt, :],
                            rhs=xsrc[:, kt % 8, :],
                            start=(kt == 0), stop=False)
                    nc.tensor.matmul(
                        ps[:, s, :], lhsT=cbias[:, m, :], rhs=ones16,
                        start=False, stop=True)
                a_sb[p] = singles.tile([P, 2, PX], fp32, tag=f"a{p}", name=f"a{p}")
                nc.scalar.copy(out=a_sb[p], in_=ps)
                a16[p] = singles.tile([P, 2, PX], fp16, tag=f"a16_{p}",
                                      name=f"a16_{p}")
                nc.vector.tensor_copy(out=a16[p], in_=ps)
                asum = work.tile([P, 2], fp32, tag="asum")
                nc.vector.tensor_reduce(asum, ps, axis=AX.X, op=Alu.add)
                asum16[p] = singles.tile([P, 2], fp16, tag=f"as16_{p}",
                                         name=f"as16_{p}")
                # 1/256 pixel-mean folded here (not into fp16 weights:
                # 0.02/256 would be subnormal in fp16)
                nc.vector.tensor_scalar_mul(asum16[p], asum, 1.0 / 256.0)

            def fc_chain(p):
                # fc1: h = relu(W1 @ avg + b1), 4 groups of 64 stacked
                hp = psB.tile([P, 2], fp32, tag="tiny8")
                for mt in (0, 1):
                    for kt in (0, 1):
                        nc.tensor.matmul(
                            hp[:, mt:mt + 1],
                            lhsT=w1v[:, p, kt, mt * P:(mt + 1) * P],
                            rhs=asum16[p][:, kt:kt + 1],
                            start=(kt == 0), stop=False)
                    nc.tensor.matmul(
                        hp[:, mt:mt + 1], lhsT=fbias[:, p, mt, :],
                        rhs=ones16[:, 0:1], start=False, stop=True)
                h_sb[p] = singles.tile([P, 2], fp16, tag=f"h{p}", name=f"h{p}")
                nc.scalar.activation(out=h_sb[p], in_=hp, func=Act.Relu)
                # per-group h with a 1.0 row so fc2's bias folds into the
                # matmul: even groups keep h in rows 0-63 (ones at row 64),
                # odd groups in rows 64-127 (ones at row 0)
                h_m[p] = singles.tile([P, 4], fp16, tag=f"hm{p}", name=f"hm{p}")
                nc.vector.memset(h_m[p], 0.0)
                for i in range(4):
                    lo = 64 * (i % 2)
                    nc.vector.tensor_copy(
                        out=h_m[p][lo:lo + 64, i:i + 1],
                        in_=h_sb[p][lo:lo + 64, i // 2:i // 2 + 1])
                    one_row = 64 if i % 2 == 0 else 0
                    nc.vector.memset(h_m[p][one_row:one_row + 1, i:i + 1], 1.0)
                # fc2: ca = sigmoid(W2 @ h + b2), all 8 columns in one bank
                cp = psB.tile([P, 2, 4], fp32, tag="tiny8")
                for s in (0, 1):
                    for i in range(4):
                        nc.tensor.matmul(
                            cp[:, s, i:i + 1], lhsT=w2v[:, p, i, s, :],
                            rhs=h_m[p][:, i:i + 1], start=True, stop=True)
                ca[p] = singles.tile([P, 2, 4], fp32, tag=f"ca{p}", name=f"ca{p}")
                nc.scalar.activation(out=ca[p], in_=cp, func=Act.Sigmoid)
                weff16[p] = singles.tile([P, 2, 4], fp16, tag=f"we{p}",
                                         name=f"we{p}")
                nc.vector.tensor_tensor(out=weff16[p], in0=ca[p],
                                        in1=sawv[:, p], op=Alu.mult)

            def saz_p(p, zs, zsum):
                sarep = [None, None]
                for i in range(4):
                    g = p + 2 * i
                    j = i % 2
                    if j == 0:
                        sarep[i // 2] = work.tile([P, 2, PX], fp16, tag="sarep",
                                                  name="sarep")
                        sps = psA.tile([P, 2, PX], fp32, tag="sa")
                    # rank-1 "broadcast" matmuls: every output partition gets
                    # the same spatial-sum row; bias via a K=1 matmul
                    for s in (0, 1):
                        nc.tensor.matmul(
                            sps[:, j, :],
                            lhsT=weff16[p][:, s, i:i + 1].to_broadcast((P, P)),
                            rhs=a16[p][:, s, :],
                            start=(s == 0), stop=False)
                    nc.tensor.matmul(
                        sps[:, j, :], lhsT=sab16[:, g, :], rhs=ones16,
                        start=False, stop=True)
                    if j == 1:
                        nc.scalar.activation(out=sarep[i // 2], in_=sps,
                                             func=Act.Sigmoid)
                for i in range(4):
                    t16 = work.tile([P, 2, PX], fp16, tag="t16")
                    for s in (0, 1):
                        # fused t = a * sa * ca in one DVE pass
                        nc.vector._custom_dve(
                            _TSA_OP, out=t16[:, s, :], in0=a16[p][:, s, :],
                            in1=sarep[i // 2][:, i % 2, :],
                            s0=ca[p][:, s, i:i + 1])
                    z_pair = zpool.tile([P, 2, PX], fp32, tag="z")
                    nc.scalar.activation(
                        out=z_pair, in_=t16, func=Act.Sigmoid,
                        accum_out=zsum[:, i:i + 1])
                    zs[i] = z_pair

            def mask_p(p, zs, zsum):
                pm = singles.tile([P, 4], fp32, tag=f"pm{p}", name=f"pm{p}")
                for i in range(4):
                    # per-group cross-partition sum via an all-ones fp32
                    # matmul: every output partition gets the group total
                    zrps = psB.tile([P, 2, 4], fp32, tag="tiny8")
                    nc.tensor.matmul(zrps[:, 0, 0:1], lhsT=ones32,
                                     rhs=zsum[:, i:i + 1], start=True, stop=True)
                    nc.vector.tensor_scalar_mul(
                        pm[:, i:i + 1], zrps[:, 0, 0:1], 1.0 / 65536.0)
                    ot = otp.tile([P, 2, PX], fp32, tag="ot")
                    # fused out = a * (1 + where(z > mean, 1, z))
                    nc.vector._custom_dve(
                        _MSK_OP, out=ot, in0=zs[i], in1=a_sb[p],
                        s0=pm[:, i:i + 1])
                    nc.sync.dma_start(
                        out=out_d.rearrange("(i x) P f -> P i x f", i=4)
                        [:, i, 2 * p:2 * p + 2, :],
                        in_=ot)

            zs0, zs1 = {}, {}
            zsum0 = singles.tile([P, 4], fp32, tag="zs0")
            zsum1 = singles.tile([P, 4], fp32, tag="zs1")
            conv_p(0)
            fc_chain(0)
            conv_p(1)
            fc_chain(1)
            saz_p(0, zs0, zsum0)
            saz_p(1, zs1, zsum1)
            mask_p(0, zs0, zsum0)
            mask_p(1, zs1, zsum1)

    nc.finalize()
    return nc


def _prep_core_inputs(x4b, w, s16, s32, cb):
    x = np.ascontiguousarray(
        x4b.reshape(KT, P, PX).transpose(1, 0, 2)).astype(np.float16)
    return {"x": x, "w": w, "s16": s16, "s32": s32, "cb": cb}


def _prep_params(cov4_w, cov4_b, fc1_w, fc1_b, fc2_w, fc2_b, sa_w, sa_b):
    f32 = np.float32
    w2d = np.asarray(cov4_w, f32).reshape(512, 2048)
    wr = w2d.reshape(MT, P, KT, P)                            # [m, mc, kt, part]
    w_arr = np.ascontiguousarray(wr.transpose(0, 3, 2, 1)).astype(np.float16)

    fc1_w = np.asarray(fc1_w, f32)
    fc1_b = np.asarray(fc1_b, f32)
    fc2_w = np.asarray(fc2_w, f32)
    fc2_b = np.asarray(fc2_b, f32)
    sa_w = np.asarray(sa_w, f32)
    sa_b = np.asarray(sa_b, f32)

    w1 = np.zeros((P, 2, 2, 256), f32)
    w2 = np.zeros((P, 2, 4, 2, P), f32)
    b1 = np.zeros((2, 2, P), f32)
    saw = np.zeros((P, 2, 2, 4), f32)
    for p in range(2):
        W1s = np.concatenate([fc1_w[p + 2 * i] for i in range(4)], axis=0)
        b1s = np.concatenate([fc1_b[p + 2 * i] for i in range(4)], axis=0)
        for kt in range(2):
            w1[:, p, kt, :] = W1s[:, kt * P:(kt + 1) * P].T
        b1[p, 0] = b1s[:P]
        b1[p, 1] = b1s[P:]
        for i in range(4):
            g = p + 2 * i
            lo = 64 * (i % 2)           # rows holding fc2 weights
            brow = 64 if i % 2 == 0 else 0
            for s in range(2):
                w2[lo:lo + 64, p, i, s, :] = fc2_w[g][s * P:(s + 1) * P, :].T
                w2[brow, p, i, s, :] = fc2_b[g, s * P:(s + 1) * P]
                saw[:, p, s, i] = sa_w[g, s * P:(s + 1) * P]

    s16 = np.zeros((P, _NSM16), np.float16)
    s16[:, _W1_OFF:_W1_OFF + 1024] = w1.reshape(P, 1024).astype(np.float16)
    s16[:, _W2_OFF:_W2_OFF + 2048] = w2.reshape(P, 2048).astype(np.float16)
    s16[0, _FB_OFF:_FB_OFF + 512] = b1.reshape(512).astype(np.float16)
    cb = np.asarray(cov4_b, f32).astype(np.float16).reshape(1, 512)

    s32 = np.zeros((P, _NSM32), f32)
    s32[:, _SAW_OFF:_SAW_OFF + 16] = saw.reshape(P, 16)
    s32[:, _SAB_OFF:_SAB_OFF + 8] = np.broadcast_to(sa_b, (P, 8))
    return w_arr, s16, s32, cb


def kernel(**inputs):
    from concourse.bass_utils import run_bass_kernel_spmd

    if "nc" not in _CACHE:
        _CACHE["nc"] = _build_program()
    nc = _CACHE["nc"]

    x4 = np.asarray(inputs["x4"], np.float32)
    B = x4.shape[0]
    w_arr, s16, s32, cb = _prep_params(
        inputs["cov4_w"], inputs["cov4_b"],
        inputs["gce_fc1_w"], inputs["gce_fc1_b"],
        inputs["gce_fc2_w"], inputs["gce_fc2_b"],
        inputs["gce_sa_w"], inputs["gce_sa_b"])

    in_maps = [
        _prep_core_inputs(x4[b].reshape(2048, PX), w_arr, s16, s32, cb)
        for b in range(B)
    ]
    res = run_bass_kernel_spmd(nc, in_maps, list(range(N_CORES)))
    _CACHE["last_results"] = res

    out = np.empty((B, 2048, 16, 16), np.float32)
    for b in range(B):
        out[b] = res.results[b]["out"].reshape(2048, 16, 16)
    return out
